# revision 14
# baseline (speedup 1.0000x reference)
"""AELoss (associative-embedding push/pull loss) on 8 TRN2 NeuronCores.

Data-parallel over batch: each of the 8 cores handles 4 images. Only the
visible (person, joint) pairs are needed (~1020 per core); the host
compacts them into [128, C] slots (C ~ 8) and the kernel issues one
[128,1] SWDGE indirect gather per slot column (the ~1us/instruction
SWDGE fixed cost makes 128-offset columns the unit of gather work; the
hardware honors one offset per partition per instruction).

Per-person [sum g, sum g^2] accumulate in PSUM via one tiny PE matmul per
column against a host-built one-hot slot->person matrix, hidden under the
remaining gathers. The post-gather tail is minimized:
  - mean subtraction folds into the Derivative_Erf activation bias
  - the pair-mask multiply and row reduction fuse into one DVE
    tensor_tensor_reduce
  - the same-image broadcast matmul runs in bf16 (single PE pass)
  - all per-image scalars (1/n, 0.5/max(n^2-n,1), sqrt(pi)/2, the -n push
    offset) are host-folded into the matmul operands, leaving one
    tensor_sub before the output DMA:
      fin[b] = [pull_b, push_b + n*inv_nn_b]

Identities (exact vs the reference):
  pull_pp = sum(g^2 v)/safe_cnt - mean^2
  pull    = pull_num / max(n,1)
  push    = (S - n) * 0.5/max(n^2-n,1),  S via sqrt(pi)/2 * D_ERF
"""

import numpy as np

B, M, K = 32, 30, 17
N = 17 * 256 * 256
NCORES = 8
BL = B // NCORES          # images per core
P = 128
PERS = BL * M             # person rows per core (120)

SQPI2 = 0.8862269254527579   # sqrt(pi)/2: D_ERF(x) = 2/sqrt(pi) exp(-x^2)

# aux (f32) column layout
C_ONEHRC = 0              # [0,30)   oneh * rc
C_NEGRC = M               # 30       -1/max(cnt,1)
C_RC = M + 1              # 31       +1/max(cnt,1)
C_SELVI = M + 2           # [32,36)  sel*valid*inv_n
C_FINSUB = M + 2 + BL     # [36,38)  rows 0..3: [0, n*inv_nn]
W_AUX = C_FINSUB + 2      # 38

# bf16 tensor column layout: wimg [0,128) | rhs1 host half [128,158) |
# last-column amat block [158,158+JLMAX) (bf16 single-pass PE for the
# tail-gating column)
C_AML = P + M
JLMAX = P
W_BF = C_AML + JLMAX

_cache = {}


def _strip_init_barrier(nc):
    """Drop the Bass-init const-AP memsets and the all-engine barrier that
    orders them — nothing in this kernel reads the const APs (activation
    bias is passed as an explicit AP)."""
    import concourse.mybir as mybir

    bb = nc.main_func.blocks[0]
    drop = set()
    for ins in bb.instructions:
        if isinstance(ins, (mybir.InstMemset, mybir.InstDrain, mybir.InstEventSemaphore)):
            drop.add(ins.name)
    if not drop:
        return
    keep = [ins for ins in bb.instructions if ins.name not in drop]
    del bb.instructions[:]
    for ins in keep:
        bb.add_instruction(ins)


def _build(C, JL):
    import concourse.bass as bass
    import concourse.bacc as bacc
    import concourse.mybir as mybir
    from concourse.tile import TileContext

    f32 = mybir.dt.float32
    bf16 = mybir.dt.bfloat16
    i32 = mybir.dt.int32
    X = mybir.AxisListType.X
    op = mybir.AluOpType

    # no partition-id loads: SPMD per-core data arrives via in_maps, nothing
    # branches on core id, and the 5 per-engine ~1us DRAM register loads the
    # id tensor costs sit right in the startup critical path
    nc = bacc.Bacc(trn_type="TRN2", enable_partition_id=False)
    _strip_init_barrier(nc)
    tags_d = nc.dram_tensor("tags", [BL * N, 1], f32, kind="ExternalInput")
    idx_d = nc.dram_tensor("idx", [P, C], i32, kind="ExternalInput")
    amat_d = nc.dram_tensor("amat", [P, C * P], f32, kind="ExternalInput")
    aux_d = nc.dram_tensor("aux", [P, W_AUX], f32, kind="ExternalInput")
    bft_d = nc.dram_tensor("bft", [P, W_BF], bf16, kind="ExternalInput")
    out_d = nc.dram_tensor("out", [BL, 2], f32, kind="ExternalOutput")

    with TileContext(nc) as tc:
        with (
            tc.tile_pool(name="sb", bufs=1) as sb,
            tc.tile_pool(name="ps", bufs=1, space="PSUM") as ps,
        ):
            # idx first: column 0 rides its own minimal DMA so the first
            # gather's wait is a 4B/partition transfer; scalar's queue opens
            # earlier than sync's (whose first dispatch sits behind a long
            # drain), so idx0 goes there and the rest rides sync
            idx0_t = sb.tile([P, 1], i32)
            nc.scalar.dma_start(out=idx0_t[:], in_=idx_d[:, 0:1])
            idxr_t = sb.tile([P, C - 1], i32)
            nc.sync.dma_start(out=idxr_t[:], in_=idx_d[:, 1:C])

            # gathers: one [128,1] indirect DMA per packed slot column;
            # bounds register hoisted so each gather skips its own MOVE.
            # (Tried one tile per column to break the apparent inter-gather
            # ordering — measured ~1.7us SLOWER; the shared tile stays.)
            br = nc.gpsimd.to_reg(BL * N - 1)
            gg = sb.tile([P, 2 * C], f32)
            for c in range(C):
                off = idx0_t[:, 0:1] if c == 0 else idxr_t[:, c - 1:c]
                nc.gpsimd.indirect_dma_start(
                    out=gg[:, 2 * c:2 * c + 1],
                    out_offset=None,
                    in_=tags_d[:],
                    in_offset=bass.IndirectOffsetOnAxis(ap=off, axis=0),
                    oob_is_err=False,
                    bounds_check=br,
                )

            # remaining small inputs on the scalar engine's HWDGE queue
            amat_t = sb.tile([P, C * P], f32)
            nc.scalar.dma_start(out=amat_t[:], in_=amat_d[:])
            aux_t = sb.tile([P, W_AUX], f32)
            nc.scalar.dma_start(out=aux_t[:], in_=aux_d[:])
            wimg_t = sb.tile([P, P], bf16)
            nc.scalar.dma_start(out=wimg_t[:], in_=bft_d[:, 0:P])
            rhs1 = sb.tile([P, 2 * M], bf16)
            nc.sync.dma_start(out=rhs1[:, M:2 * M], in_=bft_d[:, P:P + M])
            amlt = sb.tile([P, JL], bf16)
            nc.sync.dma_start(out=amlt[:], in_=bft_d[:, C_AML:C_AML + JL])

            # dummy D_ERF on DMA-ready data: pulls the ACT table load into
            # the gather window (the real activation's first dep is the
            # engine-computed bias, which would otherwise drag the ~1.3us
            # table load onto the critical path)
            dume = sb.tile([P, 1], f32)
            nc.scalar.activation(
                out=dume[:], in_=aux_t[:, C_RC:C_RC + 1],
                func=mybir.ActivationFunctionType.Derivative_Erf,
                bias=aux_t[:, C_NEGRC:C_NEGRC + 1], scale=1.0,
            )

            # per-person [sum g, sum g^2] via one accumulating matmul per
            # column: square each landed column, matmul against the one-hot
            # slot->person matrix (overlaps the remaining gathers). The LAST
            # column is the tail gate: run it in bf16 (one PE pass instead of
            # the fp32 two-pass) against a host-provided bf16 amat block —
            # only the ~124 lowest-person slots lose ~0.4% there.
            ggl_bf = sb.tile([P, 2], bf16)
            ps12 = ps.tile([P, 2], f32)
            for c in range(C):
                if c == C - 1:
                    # bf16 (g, g^2) pair for the last column: two tiny DVE
                    # writes, then a single-pass bf16 matmul
                    nc.vector.tensor_scalar(
                        out=ggl_bf[:, 0:1], in0=gg[:, 2 * c:2 * c + 1],
                        scalar1=1.0, scalar2=None, op0=op.mult,
                    )
                    nc.vector.tensor_mul(
                        out=ggl_bf[:, 1:2],
                        in0=gg[:, 2 * c:2 * c + 1], in1=gg[:, 2 * c:2 * c + 1],
                    )
                    nc.tensor.matmul(
                        out=ps12[:JL, :], lhsT=amlt[:, 0:JL],
                        rhs=ggl_bf[:], start=False, stop=True,
                        skip_group_check=True,
                    )
                else:
                    nc.vector.tensor_mul(
                        out=gg[:, 2 * c + 1:2 * c + 2],
                        in0=gg[:, 2 * c:2 * c + 1], in1=gg[:, 2 * c:2 * c + 1],
                    )
                    nc.tensor.matmul(
                        out=ps12[:, :], lhsT=amat_t[:, c * P:c * P + P],
                        rhs=gg[:, 2 * c:2 * c + 2],
                        start=(c == 0), stop=False,
                        skip_group_check=True,
                    )

            # ---- post-gather tail (reads the PSUM sums directly) ----
            # rhs1 cols 0:30 = oneh*rc*sum_g (bf16 for the broadcast matmul)
            nc.vector.tensor_tensor(
                out=rhs1[:, 0:M], in0=aux_t[:, C_ONEHRC:C_ONEHRC + M],
                in1=ps12[:, 0:1].to_broadcast([P, M]), op=op.mult,
            )
            negm = sb.tile([P, 1], f32)
            nc.vector.tensor_scalar(
                out=negm[:], in0=ps12[:, 0:1],
                scalar1=aux_t[:, C_NEGRC:C_NEGRC + 1], scalar2=None, op0=op.mult,
            )
            # msq on DVE: the ACT engine must stay free for the
            # critical-path D_ERF right after the m2 matmul (tried Square on
            # ACT — it delayed D_ERF by ~430ns)
            msq = sb.tile([P, 1], f32)
            nc.vector.tensor_mul(out=msq[:], in0=negm[:], in1=negm[:])
            # rhs2 col0: pv = rc*sum_g2 - mean^2  (pull_pp, pre-valid)
            rhs2 = sb.tile([P, 2], f32)
            nc.vector.tensor_scalar(
                out=rhs2[:, 0:1], in0=ps12[:, 1:2],
                scalar1=aux_t[:, C_RC:C_RC + 1], scalar2=msq[:],
                op0=op.mult, op1=op.subtract,
            )

            # same-image broadcast: m2[:,0:30]=means, m2[:,30:60]=mask*ratio
            m2 = ps.tile([P, 2 * M], f32)
            nc.tensor.matmul(
                out=m2[:], lhsT=wimg_t[:], rhs=rhs1[:], start=True, stop=True
            )

            # e = 2/sqrt(pi) * exp(-(mean_j - mean_p)^2), bias folds the sub
            e = sb.tile([P, M], f32)
            nc.scalar.activation(
                out=e[:], in_=m2[:, 0:M],
                func=mybir.ActivationFunctionType.Derivative_Erf,
                bias=negm[:, 0:1], scale=1.0,
            )
            # rhs2 col1: rowsum of masked e (mask carries valid_j*ratio*sqpi2),
            # fused mult+reduce in one DVE pass
            em = sb.tile([P, M], f32)
            nc.vector.scalar_tensor_tensor(
                out=em[:], in0=e[:], scalar=1.0, in1=m2[:, M:2 * M],
                op0=op.mult, op1=op.mult, accum_out=rhs2[:, 1:2],
            )

            # fin[b] = [pull_b, push_b + n*inv_nn_b]
            fin = ps.tile([BL, 2], f32)
            nc.tensor.matmul(
                out=fin[:], lhsT=aux_t[:, C_SELVI:C_SELVI + BL], rhs=rhs2[:],
                start=True, stop=True,
            )
            outt = sb.tile([BL, 2], f32)
            nc.vector.tensor_sub(
                out=outt[:], in0=fin[:], in1=aux_t[0:BL, C_FINSUB:C_FINSUB + 2]
            )
            nc.sync.dma_start(out=out_d[:], in_=outt[:])

    nc.compile()
    return nc


def _in_maps(tags, joints):
    import ml_dtypes

    tags = np.ascontiguousarray(np.asarray(tags, dtype=np.float32)).reshape(B, N)
    joints = np.asarray(joints, dtype=np.int32)
    idx_all = joints[..., 0]                               # [B, M, K]
    vis_all = joints[..., 1] > 0                           # [B, M, K] bool

    # balance images across cores so the max per-core visible-slot count
    # (which sets the gather-column count C) is minimized: LPT bin packing
    vis_cnt = vis_all.sum(axis=(1, 2))
    bins = [[] for _ in range(NCORES)]
    sums = [0] * NCORES
    for i in np.argsort(-vis_cnt):
        c = min((b for b in range(NCORES) if len(bins[b]) < BL),
                key=lambda b: sums[b])
        bins[c].append(int(i))
        sums[c] += int(vis_cnt[i])
    assign = [sorted(b) for b in bins]                     # [NCORES][BL] image ids

    # compact visible (person, joint) slots per core
    per_core = []
    C = 1
    for c in range(NCORES):
        persons = []
        fidx = []
        for b, img in enumerate(assign[c]):
            vb = vis_all[img]                              # [M, K]
            mm, kk = np.nonzero(vb)
            persons.append(b * M + mm)
            fidx.append(idx_all[img][mm, kk] + b * N)
        persons = np.concatenate(persons)
        fidx = np.concatenate(fidx)
        order = np.argsort(-persons, kind="stable")
        persons, fidx = persons[order], fidx[order]
        per_core.append((persons, fidx))
        C = max(C, (len(fidx) + P - 1) // P)

    # slots are person-descending, so the last column only involves the
    # lowest person ids: its A matmul can use a narrow LDWEIGHTS
    JL = 1
    lo = (C - 1) * P
    for persons, fidx in per_core:
        if len(fidx) > lo:
            JL = max(JL, int(persons[lo:].max()) + 1)
    JL = min(P, ((JL + 15) // 16) * 16)

    pp = np.arange(P)
    mrow = pp < PERS
    wimg = ((pp[:, None] // M == pp[None, :] // M)
            & mrow[:, None] & mrow[None, :]).astype(np.float32)

    in_maps = []
    for c in range(NCORES):
        persons, fidx = per_core[c]
        n_slots = len(fidx)
        idx_l = np.zeros((P, C), np.int32)
        amat = np.zeros((P, C * P), np.float32)
        s = np.arange(n_slots)
        sp, scol = s % P, s // P
        idx_l[sp, scol] = fidx
        amat[sp, scol * P + persons] = 1.0

        aux = np.zeros((P, W_AUX), np.float32)
        bft = np.zeros((P, W_BF), np.float32)
        bft[:, 0:P] = wimg
        # bf16 copy of the last amat column block (the tail-gating matmul)
        bft[:, C_AML:C_AML + JL] = amat[:, (C - 1) * P:(C - 1) * P + JL]
        for b, img in enumerate(assign[c]):
            cnt = vis_all[img].sum(-1)                     # [M]
            valid = cnt > 0
            nb = int(valid.sum())
            inv_n = 1.0 / max(nb, 1)
            inv_nn = 0.5 / max(nb * nb - nb, 1)
            ratio = inv_nn / inv_n
            rows = slice(b * M, (b + 1) * M)
            rcp = 1.0 / np.maximum(cnt, 1)
            aux[rows, C_NEGRC] = -rcp
            aux[rows, C_RC] = rcp
            aux[pp[rows], C_ONEHRC + pp[rows] % M] = rcp
            aux[rows, C_SELVI + b] = valid * inv_n
            aux[b, C_FINSUB + 1] = nb * inv_nn
            bft[pp[rows], P + pp[rows] % M] = valid * (SQPI2 * ratio)
        in_maps.append({
            "tags": np.ascontiguousarray(tags[assign[c]].reshape(BL * N, 1)),
            "idx": idx_l,
            "amat": amat,
            "aux": aux,
            "bft": bft.astype(ml_dtypes.bfloat16),
        })
    return (C, JL), in_maps, assign


def _run(key, in_maps, trace=False):
    from concourse import bass_utils

    if key not in _cache:
        _cache[key] = _build(*key)
    return bass_utils.run_bass_kernel_spmd(
        _cache[key], in_maps, core_ids=list(range(NCORES)), trace=trace
    )


def kernel(tags, joints):
    key, in_maps, assign = _in_maps(tags, joints)
    res = _run(key, in_maps)
    push = np.zeros(B, np.float32)
    pull = np.zeros(B, np.float32)
    for c in range(NCORES):
        o = res.results[c]["out"]
        for b, img in enumerate(assign[c]):
            pull[img] = o[b, 0]
            push[img] = o[b, 1]
    return push, pull

